# revision 1
# baseline (speedup 1.0000x reference)
"""Attention-LSTM captioning RNN on 8 Trainium2 NeuronCores.

Data-parallel over batch N=128 -> 16 samples/core.  Per-core kernel:
  phase 1: xw[n,t,:] = x[n,t,:] @ Wx + b           (dense precompute)
  phase 2: G2[(l,n),j] = sum_h Af[n,h,l] Wattn[h,j] (folds attn@Wattn
           into a 16-long contraction against softmax weights)
  phase 3: 64 recurrent steps:
           scores -> softmax -> sparse-w -> a = xw + h@Wh + w.G2
           gates -> c,h; h transposed (DVE 32x32) for next step.

Matmuls are bf16 with f32 PSUM accumulation; cell state and softmax
are f32.  The thin batch (M=16) is packed 4-wide into the PE array via
tile_position column groups, giving the gate layout: partition 32q+n
holds sample n, j-columns [g*1280 + q*320, +320) for gate g.
"""

import sys

if "/opt/trn_rl_repo" not in sys.path:
    sys.path.insert(0, "/opt/trn_rl_repo")

import numpy as np

import concourse.bass as bass
import concourse.bacc as bacc
import concourse.mybir as mybir
from concourse import tile
from concourse.bass_utils import run_bass_kernel_spmd

N_CORES = 8
NL = 16          # samples per core
T = 64
D = 512
H = 1280
FH = 4 * H       # 5120
L = 16           # 4x4 spatial locations
NT = NL * T      # 1024
CH = 320         # per-(gate, colgroup) j-chunk:  FH = 4 gates * 4 groups * 320
F32 = mybir.dt.float32
BF16 = mybir.dt.bfloat16
_BF16_NP = mybir.dt.np(BF16)
KH = H // 128    # 10 contraction tiles over H


def _ap(t, dims):
    a = t[:]
    return bass.AP(a.tensor, a.offset, dims)


def _bcast_l(ap, l):
    """(P, F) AP -> (P, [l x stride-0, F])."""
    return bass.AP(ap.tensor, ap.offset, [ap.ap[0], [0, l]] + ap.ap[1:])


def _split16(ap):
    """(P, 256) AP -> (P, 16, 16)."""
    p, (s, c) = ap.ap[0], ap.ap[-1]
    assert c == 256 and s == 1
    return bass.AP(ap.tensor, ap.offset, [p, [16, 16], [1, 16]])


def _qn_view(t):
    """(128, F) tile -> (q 4, n 16, F) AP over partitions 32q+n, n<16."""
    a = t[:]
    pitch = a.ap[0][0]
    f = a.ap[-1]
    return bass.AP(a.tensor, a.offset, [[32 * pitch, 4], [pitch, 16], f])


def build_nc(t_steps=T, n_cores=N_CORES, debug=False):
    nc = bacc.Bacc(
        "TRN2",
        target_bir_lowering=False,
        debug=False,
        enable_asserts=False,
        num_devices=n_cores,
    )

    xT_d = nc.dram_tensor("xT", [D, NT], BF16, kind="ExternalInput")
    afp_d = nc.dram_tensor("afp", [128, KH * L * NL], BF16, kind="ExternalInput")
    wx_d = nc.dram_tensor("wx", [D, FH], BF16, kind="ExternalInput")
    wh_d = nc.dram_tensor("wh", [H, FH], BF16, kind="ExternalInput")
    wat_d = nc.dram_tensor("wat", [H, FH], BF16, kind="ExternalInput")
    bcols_d = nc.dram_tensor("bcols", [128, FH], F32, kind="ExternalInput")
    h0t2_d = nc.dram_tensor("h0t2", [128, CH], BF16, kind="ExternalInput")
    c0g_d = nc.dram_tensor("c0g", [128, CH], F32, kind="ExternalInput")
    m16b_d = nc.dram_tensor("m16b", [128, 16], BF16, kind="ExternalInput")
    m16f_d = nc.dram_tensor("m16f", [128, 16], F32, kind="ExternalInput")
    mTf_d = nc.dram_tensor("mTf", [16, 128], F32, kind="ExternalInput")
    ones_d = nc.dram_tensor("ones", [128, 1], BF16, kind="ExternalInput")
    y_d = nc.dram_tensor("y", [t_steps, 128, CH], F32, kind="ExternalOutput")
    if debug:
        dbg_g2_d = nc.dram_tensor("dbg_g2", [2, 128, FH], BF16, kind="ExternalOutput")
        dbg_xw_d = nc.dram_tensor("dbg_xw", [4, 128, CH], F32, kind="ExternalOutput")
        dbg_a_d = nc.dram_tensor("dbg_a", [4, 128, CH], F32, kind="ExternalOutput")
        dbg_w_d = nc.dram_tensor("dbg_w", [2, 128, 16], BF16, kind="ExternalOutput")
        dbg_ht_d = nc.dram_tensor("dbg_ht", [128, CH], BF16, kind="ExternalOutput")
        dbg_s_d = nc.dram_tensor("dbg_s", [2, 128, 2], F32, kind="ExternalOutput")
    xw_d = nc.dram_tensor("xw_scratch", [T, 128, 4 * CH], F32)
    junk_d = nc.dram_tensor("junk_out", [1, 64], F32)

    inv_sqrt_h = 1.0 / float(np.sqrt(H))
    act = mybir.ActivationFunctionType

    with tile.TileContext(nc) as tc:
        with (
            tc.tile_pool(name="persist", bufs=1) as pp,
            tc.tile_pool(name="state", bufs=1) as st,
            tc.tile_pool(name="psA", bufs=1, space="PSUM") as psA,
            tc.tile_pool(name="psS", bufs=1, space="PSUM") as psS,
        ):
            # ---------- constants / persistents ---------------------------
            afp = pp.tile([128, KH * L * NL], BF16, tag="afp", name="afp")
            nc.sync.dma_start(afp[:], afp_d[:, :])
            m16b = pp.tile([128, 16], BF16, tag="m16b", name="m16b")
            m16f = pp.tile([128, 16], F32, tag="m16f", name="m16f")
            mTf = pp.tile([16, 128], F32, tag="mTf", name="mTf")
            ones = pp.tile([128, 1], BF16, tag="ones", name="ones")
            nc.sync.dma_start(m16b[:], m16b_d[:, :])
            nc.sync.dma_start(m16f[:], m16f_d[:, :])
            nc.sync.dma_start(mTf[:], mTf_d[:, :])
            nc.sync.dma_start(ones[:], ones_d[:, :])

            # ---------- phase 1: xw = x @ Wx + b --------------------------
            with tc.tile_pool(name="ph1", bufs=1) as p1, \
                 tc.tile_pool(name="ph1w", bufs=2) as p1w:
                xTs = [p1.tile([128, NT], BF16, tag=f"xT{k}", name=f"xT{k}") for k in range(D // 128)]
                for k in range(D // 128):
                    nc.sync.dma_start(xTs[k][:], xT_d[128 * k : 128 * (k + 1), :])
                wxs = [p1.tile([128, FH], BF16, tag=f"wx{k}", name=f"wx{k}") for k in range(D // 128)]
                for k in range(D // 128):
                    nc.sync.dma_start(wxs[k][:], wx_d[128 * k : 128 * (k + 1), :])
                bcols = p1.tile([128, FH], F32, tag="bcols", name="bcols")
                nc.sync.dma_start(bcols[:], bcols_d[:, :])

                for m in range(NT // 128):          # 8 row tiles of (n,t)
                    xwrow = p1w.tile([128, FH], F32, tag="xwrow", name="xwrow")
                    for cc in range(FH // CH):      # 16 col chunks of 320
                        r, q = cc // 4, cc % 4
                        ps = psA.tile([128, 512], F32, tag=f"a{cc % 4}", name=f"a{cc % 4}")
                        for k in range(D // 128):
                            nc.tensor.matmul(
                                ps[:, 0:CH],
                                xTs[k][:, 128 * m : 128 * (m + 1)],
                                wxs[k][:, CH * cc : CH * (cc + 1)],
                                start=(k == 0),
                                stop=(k == D // 128 - 1),
                            )
                        nc.vector.tensor_add(
                            xwrow[:, CH * cc : CH * (cc + 1)],
                            ps[:, 0:CH],
                            bcols[:, CH * cc : CH * (cc + 1)],
                        )
                    # src free j = (r,q,h); dst xw_d[t, 32q+n, r*320+h]
                    for i in range(2):
                        for r in range(4):
                            sview = xwrow[64 * i : 64 * (i + 1),
                                          4 * CH * r : 4 * CH * (r + 1)]
                            dst = bass.AP(
                                xw_d[:, :, :].tensor,
                                (2 * m + i) * (4 * CH) + CH * r,
                                [[128 * 4 * CH, 64], [32 * 4 * CH, 4], [1, CH]],
                            )
                            nc.sync.dma_start(dst, sview)

            # ---------- phase 2: G2 = afp.T @ Wattn -----------------------
            g2 = [pp.tile([128, FH], BF16, tag=f"g2_{m2}", name=f"g2_{m2}") for m2 in range(2)]
            with tc.tile_pool(name="ph2", bufs=1) as p2:
                wats = [p2.tile([128, FH], BF16, tag=f"wat{k}", name=f"wat{k}") for k in range(KH)]
                for k in range(KH):
                    nc.sync.dma_start(wats[k][:], wat_d[128 * k : 128 * (k + 1), :])
                for m2 in range(2):
                    for jj in range(FH // 512):
                        ps = psA.tile([128, 512], F32, tag=f"a{(2 * jj + m2) % 4}", name=f"a{(2 * jj + m2) % 4}")
                        for k in range(KH):
                            nc.tensor.matmul(
                                ps[:],
                                afp[:, 256 * k + 128 * m2 : 256 * k + 128 * (m2 + 1)],
                                wats[k][:, 512 * jj : 512 * (jj + 1)],
                                start=(k == 0),
                                stop=(k == KH - 1),
                            )
                        nc.vector.tensor_copy(
                            g2[m2][:, 512 * jj : 512 * (jj + 1)], ps[:]
                        )
                if debug:
                    for m2 in range(2):
                        nc.sync.dma_start(dbg_g2_d[m2, :, :], g2[m2][:])

            # ---------- Wh resident + recurrence pools --------------------
            rec_pools = tc.tile_pool(name="whp", bufs=1)
            whp = rec_pools.__enter__()
            wk_cm = tc.tile_pool(name="wk", bufs=2)
            wk = wk_cm.__enter__()
            pkp_cm = tc.tile_pool(name="pkp", bufs=3)
            pkp = pkp_cm.__enter__()
            whs = [whp.tile([128, FH], BF16, tag=f"wh{k}", name=f"wh{k}") for k in range(KH)]
            for k in range(KH):
                nc.sync.dma_start(whs[k][:], wh_d[128 * k : 128 * (k + 1), :])

            # ---------- state ---------------------------------------------
            hT2x = [st.tile([128, CH], BF16, tag=f"hT2x{i}", name=f"hT2x{i}") for i in range(2)]
            nc.sync.dma_start(hT2x[0][:], h0t2_d[:, :])
            cg = st.tile([128, CH], F32, tag="cg", name="cg")
            nc.sync.dma_start(cg[:], c0g_d[:, :])

            # ---------- phase 3: recurrence -------------------------------
            psJ = psS.tile([1, 64], F32, tag="psJ", name="psJ")
            for t in range(t_steps):
                jstart = (t == 0)
                hT = hT2x[t % 2]
                hTn = hT2x[(t + 1) % 2]

                xwt = wk.tile([128, 4 * CH], F32, tag="xwt", name="xwt")
                nc.sync.dma_start(xwt[:], xw_d[t, :, :])

                # ---- attention scores -> sparse softmax weights ----
                psm0 = psS.tile([128, 8], F32, tag="psm0", name="psm0")
                psm1 = psS.tile([128, 8], F32, tag="psm1", name="psm1")
                ps_s = [psm0[:, 0:1], psm1[:, 0:1]]
                p2 = pkp.tile([128, KH * L * NL], BF16, tag="pk", name="pk")
                pa = p2[:]
                aa = afp[:]
                ha = hT[:]
                nc.vector.tensor_mul(
                    bass.AP(pa.tensor, pa.offset, [pa.ap[0], [256, KH], [16, 16], [1, 16]]),
                    bass.AP(aa.tensor, aa.offset, [aa.ap[0], [256, KH], [16, 16], [1, 16]]),
                    bass.AP(ha.tensor, ha.offset, [ha.ap[0], [32, KH], [0, 16], [1, 16]]),
                )
                for k in range(KH):
                    for m2 in range(2):
                        nc.tensor.matmul(
                            ps_s[m2],
                            p2[:, 256 * k + 128 * m2 : 256 * k + 128 * (m2 + 1)],
                            ones[:],
                            start=(k == 0),
                            stop=(k == KH - 1),
                        )
                if debug and t == 0:
                    nc.sync.dma_start(dbg_ht_d[:, :], hT[:])
                expv = []
                for m2 in range(2):
                    e = wk.tile([128, 1], F32, tag=f"exp{m2}", name=f"exp{m2}")
                    nc.scalar.activation(
                        e[:], ps_s[m2], act.Exp, scale=inv_sqrt_h
                    )
                    expv.append(e)
                ps_d = psm0[0:16, 4:5]
                for m2 in range(2):
                    nc.tensor.matmul(
                        ps_d, m16f[:], expv[m2][:],
                        start=(m2 == 0), stop=(m2 == 1),
                    )
                rden = wk.tile([16, 1], F32, tag="rden", name="rden")
                nc.vector.reciprocal(rden[:], ps_d)
                ps_r = psm0[:, 6:7]
                nc.tensor.matmul(ps_r, mTf[:], rden[:], start=True, stop=True)
                wsparse = []
                for m2 in range(2):
                    v = wk.tile([128, 1], BF16, tag=f"v{m2}", name=f"v{m2}")
                    nc.vector.tensor_mul(v[:], expv[m2][:], ps_r)
                    w_sp = wk.tile([128, 16], BF16, tag=f"wsp{m2}", name=f"wsp{m2}")
                    vb = v[:]
                    nc.vector.tensor_mul(
                        w_sp[:], m16b[:],
                        bass.AP(vb.tensor, vb.offset, [vb.ap[0], [0, 16]]),
                    )
                    wsparse.append(w_sp)
                if debug and t == 0:
                    for m2 in range(2):
                        nc.sync.dma_start(dbg_w_d[m2, :, :], wsparse[m2][:])
                        se = wk.tile([128, 2], F32, tag=f"dbgs{m2}", name=f"dbgs{m2}")
                        nc.vector.tensor_copy(se[:, 0:1], ps_s[m2])
                        nc.vector.tensor_copy(se[:, 1:2], expv[m2][:])
                        nc.sync.dma_start(dbg_s_d[m2, :, :], se[:])

                # ---- a = h @ Wh + w . G2  (4 gates x 4 colgroups) ----
                psa = [psA.tile([128, 512], F32, tag=f"a{r}", name=f"a{r}") for r in range(4)]
                for r in range(4):
                    for k in range(KH):
                        for q in range(4):
                            cc = 4 * r + q
                            nc.tensor.matmul(
                                psa[r][32 * q : 32 * q + 16, 0:CH],
                                hT[:, 32 * k : 32 * k + 16],
                                whs[k][:, CH * cc : CH * (cc + 1)],
                                start=(k == 0),
                                stop=False,
                                tile_position=(0, 32 * q),
                                skip_group_check=True,
                            )
                    for m2 in range(2):
                        for q in range(4):
                            cc = 4 * r + q
                            nc.tensor.matmul(
                                psa[r][32 * q : 32 * q + 16, 0:CH],
                                wsparse[m2][:],
                                g2[m2][:, CH * cc : CH * (cc + 1)],
                                start=False,
                                stop=(m2 == 1),
                                tile_position=(0, 32 * q),
                                skip_group_check=True,
                            )

                # ---- gates ----
                ga = []
                for r in range(4):
                    a = wk.tile([128, CH], F32, tag=f"ga{r}", name=f"ga{r}")
                    nc.vector.tensor_add(a[:], psa[r][:, 0:CH], xwt[:, CH * r : CH * (r + 1)])
                    ga.append(a)
                if debug and t == 0:
                    for r in range(4):
                        nc.sync.dma_start(dbg_a_d[r, :, :], ga[r][:])
                        nc.sync.dma_start(dbg_xw_d[r, :, :], xwt[:, CH * r : CH * (r + 1)])
                alu = mybir.AluOpType
                t_i = wk.tile([128, CH], F32, tag="t_i", name="t_i")
                t_f = wk.tile([128, CH], F32, tag="t_f", name="t_f")
                t_o = wk.tile([128, CH], F32, tag="t_o", name="t_o")
                tg = wk.tile([128, CH], F32, tag="tg", name="tg")
                nc.scalar.activation(t_i[:], ga[0][:], act.Tanh, scale=0.5)
                nc.scalar.activation(t_f[:], ga[1][:], act.Tanh, scale=0.5)
                nc.tensor.matmul(psJ[0:1, :], ga[1][:, 0:1], xwt[:, 0:64],
                                 start=jstart, stop=False, skip_group_check=True)
                nc.scalar.activation(t_o[:], ga[2][:], act.Tanh, scale=0.5)
                nc.scalar.activation(tg[:], ga[3][:], act.Tanh)
                # sigma(x) = (tanh(x/2)+1)/2
                t1 = wk.tile([128, CH], F32, tag="t1", name="t1")
                nc.vector.scalar_tensor_tensor(
                    t1[:], t_f[:], 1.0, cg[:], alu.add, alu.mult)
                t2 = wk.tile([128, CH], F32, tag="t2", name="t2")
                nc.vector.scalar_tensor_tensor(
                    t2[:], t_i[:], 1.0, tg[:], alu.add, alu.mult)
                nc.vector.tensor_add(t1[:], t1[:], t2[:])
                nc.vector.tensor_scalar_mul(cg[:], t1[:], 0.5)
                tc_ = wk.tile([128, CH], F32, tag="tc", name="tc")
                nc.scalar.activation(tc_[:], cg[:], act.Tanh)
                nc.tensor.matmul(psJ[0:1, :], tc_[:, 0:1], xwt[:, 0:64],
                                 start=False, stop=False, skip_group_check=True)
                u = wk.tile([128, CH], F32, tag="u", name="u")
                nc.vector.scalar_tensor_tensor(
                    u[:], t_o[:], 1.0, tc_[:], alu.add, alu.mult)
                hf = wk.tile([128, CH], F32, tag="hf", name="hf")
                nc.vector.tensor_scalar_mul(hf[:], u[:], 0.5)
                hb = wk.tile([128, CH], BF16, tag="hb", name="hb")
                nc.vector.tensor_scalar_mul(hb[:], u[:], 0.5)
                nc.tensor.matmul(psJ[0:1, :], hb[:, 0:1], g2[0][:, 0:64],
                                 start=False, stop=False, skip_group_check=True)

                nc.sync.dma_start(y_d[t, :, :], hf[:])

                if t + 1 < t_steps:
                    for q in range(4):
                        for j in range(CH // 32):
                            hcol = CH * q + 32 * j
                            k, p0 = hcol // 128, hcol % 128
                            nc.vector.transpose(
                                hTn[p0 : p0 + 32, 32 * k : 32 * k + 32],
                                hb[32 * q : 32 * q + 32, 32 * j : 32 * j + 32],
                            )
                        jc = [0, 64, 160, 224][q]
                        nc.tensor.matmul(
                            psJ[0:1, :], hTn[:, jc : jc + 1],
                            g2[0][:, 0:64],
                            start=False, stop=False,
                            skip_group_check=True,
                        )

            jout = wk.tile([1, 64], F32, tag="jout", name="jout")
            nc.vector.tensor_copy(jout[:], psJ[:])
            nc.sync.dma_start(junk_d[:, :], jout[:])
            pkp_cm.__exit__(None, None, None)
            wk_cm.__exit__(None, None, None)
            rec_pools.__exit__(None, None, None)

    nc.compile()
    return nc


_NC_CACHE = {}


def _get_nc(t_steps=T):
    if t_steps not in _NC_CACHE:
        _NC_CACHE[t_steps] = build_nc(t_steps)
    return _NC_CACHE[t_steps]


def _prep_core_inputs(x, A, Wx, Wh, Wattn, b, c, t_steps=T):
    n0, n1 = NL * c, NL * (c + 1)
    xl = x[n0:n1]                                # (16, T, D)
    Afl = A[n0:n1].reshape(NL, H, L)             # (16, H, 16)
    h0 = Afl.mean(axis=-1).astype(np.float32)    # (16, H)

    xT = np.zeros((D, NL, T), np.float32)
    xT[:, :, :t_steps] = xl[:, :t_steps].transpose(2, 0, 1)
    xT = xT.reshape(D, NT)
    afp = np.ascontiguousarray(Afl.transpose(1, 2, 0).reshape(H, L * NL))
    afp = np.ascontiguousarray(
        afp.reshape(KH, 128, L * NL).transpose(1, 0, 2).reshape(128, KH * L * NL)
    )
    h0t2 = np.zeros((128, CH), np.float32)
    ht = h0.T  # (H, 16)
    for k in range(KH):
        h0t2[:, 32 * k : 32 * k + 16] = ht[128 * k : 128 * (k + 1), :]
    c0g = np.zeros((128, CH), np.float32)
    c0g.reshape(4, 32, CH)[:, :16, :] = h0.reshape(NL, 4, CH).transpose(1, 0, 2)

    p = np.arange(128)
    m16 = (p[:, None] % 16 == np.arange(16)[None, :]).astype(np.float32)
    bcols = np.broadcast_to(b.astype(np.float32), (128, FH)).copy()

    bf = _BF16_NP
    return {
        "xT": xT.astype(bf),
        "afp": afp.astype(bf),
        "wx": np.asarray(Wx, np.float32).astype(bf),
        "wh": np.asarray(Wh, np.float32).astype(bf),
        "wat": np.asarray(Wattn, np.float32).astype(bf),
        "bcols": bcols,
        "h0t2": h0t2.astype(bf),
        "c0g": c0g,
        "m16b": m16.astype(bf),
        "m16f": m16,
        "mTf": np.ascontiguousarray(m16.T),
        "ones": np.ones((128, 1), bf),
    }


def _run(x, A, Wx, Wh, Wattn, b, t_steps=T, trace=False):
    nc = _get_nc(t_steps)
    x = np.asarray(x, np.float32)
    A = np.asarray(A, np.float32)
    in_maps = [
        _prep_core_inputs(x, A, Wx, Wh, Wattn, b, c, t_steps)
        for c in range(N_CORES)
    ]
    kw = {}
    if trace:
        import types
        try:
            import antenv.axon_hooks  # noqa: F401
        except ImportError:
            from trn_agent_boot.trn_boot import _ntff_profile_via_ctypes
            hook = _ntff_profile_via_ctypes("/opt/axon/libaxon_pjrt.so")
            mod = types.ModuleType("antenv.axon_hooks")
            mod.get_axon_ntff_profile_hook = lambda: hook
            sys.modules["antenv.axon_hooks"] = mod
        kw["trace"] = True
    res = run_bass_kernel_spmd(nc, in_maps, core_ids=list(range(N_CORES)), **kw)
    outs = []
    for r in res.results:
        y2 = r["y"].reshape(t_steps, 4, 32, CH)[:, :, :NL, :]
        outs.append(np.ascontiguousarray(y2.transpose(2, 0, 1, 3).reshape(NL, t_steps, H)))
    return np.concatenate(outs, axis=0), res.exec_time_ns


def kernel(x, A, Wx, Wh, Wattn, b):
    out, _ = _run(x, A, Wx, Wh, Wattn, b)
    return out



# revision 4
# speedup vs baseline: 1.7020x; 1.7020x over previous
"""Attention-LSTM captioning RNN on 8 Trainium2 NeuronCores.

Data-parallel over batch N=128 -> 16 samples/core.  Per-core kernel:
  phase 1: xw[t,n,:] = x[n,t,:] @ Wx + b      (dense precompute, bf16,
           stored t-major [T,16,FH] so it can be PSUM-injected later)
  phase 2: G2[(l,n),j] = sum_h Af[n,h,l] Wattn[h,j]
  phase 3: 64 recurrent steps:
           scores -> softmax -> sparse-w
           a = xw (identity-matmul inject) + h@Wh + w.G2   (PSUM)
           gates -> c,h

Key layout trick: gate columns are interleaved host-side so that
partition-group q owns h-dims {128b+32q+c}.  Then the per-step h
transpose (gate layout -> hT layout) is a SINGLE blocked 32x32 DVE
stream-transpose of [128,320] instead of 40 separate 32x32 ops.

State is kept as D=2c and hT holds 2h (Wh pre-scaled by 1/2), which
lets every sigmoid/tanh scale fold into ACT's input scale; y output
is 2h in bf16, halved on the host.

Junk matmuls pad the PE queue through the per-step vector tail so the
PE p-state (2.4GHz after 3us continuous busy) never resets.
"""

import sys

if "/opt/trn_rl_repo" not in sys.path:
    sys.path.insert(0, "/opt/trn_rl_repo")

import numpy as np

import concourse.bass as bass
import concourse.bacc as bacc
import concourse.mybir as mybir
from concourse import tile
from concourse.bass_utils import run_bass_kernel_spmd

N_CORES = 8
NL = 16          # samples per core
T = 64
D = 512
H = 1280
FH = 4 * H       # 5120
L = 16           # 4x4 spatial locations
CH = 320         # per-(gate, colgroup) j-chunk
F32 = mybir.dt.float32
BF16 = mybir.dt.bfloat16
_BF16_NP = mybir.dt.np(BF16)
KH = H // 128    # 10 contraction tiles over H
NJUNK = 8        # keep-warm matmuls per step

# gate storage index (reference split order): i=0, f=1, o=2, g=3
GI, GF, GO, GG = 0, 1, 2, 3


def build_nc(t_steps=T, n_cores=N_CORES):
    assert t_steps % 8 == 0
    NT = NL * t_steps
    nc = bacc.Bacc(
        "TRN2",
        target_bir_lowering=False,
        debug=False,
        enable_asserts=False,
        num_devices=n_cores,
    )

    xT_d = nc.dram_tensor("xT", [D, NT], BF16, kind="ExternalInput")
    afp_d = nc.dram_tensor("afp", [128, KH * L * NL], BF16, kind="ExternalInput")
    wx_d = nc.dram_tensor("wx", [D, FH], BF16, kind="ExternalInput")
    wh_d = nc.dram_tensor("wh", [H, FH], BF16, kind="ExternalInput")
    wat_d = nc.dram_tensor("wat", [H, FH], BF16, kind="ExternalInput")
    bcols_d = nc.dram_tensor("bcols", [128, FH], F32, kind="ExternalInput")
    h0t2_d = nc.dram_tensor("h0t2", [128, CH], BF16, kind="ExternalInput")
    c0g_d = nc.dram_tensor("c0g", [128, CH], F32, kind="ExternalInput")
    m16b_d = nc.dram_tensor("m16b", [128, 16], BF16, kind="ExternalInput")
    m16f_d = nc.dram_tensor("m16f", [128, 16], F32, kind="ExternalInput")
    mTf_d = nc.dram_tensor("mTf", [16, 128], F32, kind="ExternalInput")
    ones_d = nc.dram_tensor("ones", [128, 1], BF16, kind="ExternalInput")
    i16p_d = nc.dram_tensor("i16p", [128, 16], BF16, kind="ExternalInput")
    y_d = nc.dram_tensor("y", [t_steps, 128, CH], BF16, kind="ExternalOutput")
    xwT_d = nc.dram_tensor("xwT_scratch", [t_steps, NL, FH], BF16)
    junk_d = nc.dram_tensor("junk_out", [1, 64], F32)

    is2 = 0.5 / float(np.sqrt(H))      # exp scale (hT carries 2h)
    act = mybir.ActivationFunctionType
    alu = mybir.AluOpType

    with tile.TileContext(nc) as tc:
        with (
            tc.tile_pool(name="persist", bufs=1) as pp,
            tc.tile_pool(name="state", bufs=1) as st,
            tc.tile_pool(name="psA", bufs=1, space="PSUM") as psA,
            tc.tile_pool(name="psS", bufs=1, space="PSUM") as psS,
        ):
            # ---------- constants / persistents ---------------------------
            afp = pp.tile([128, KH * L * NL], BF16, tag="afp", name="afp")
            nc.sync.dma_start(afp[:], afp_d[:, :])
            m16b = pp.tile([128, 16], BF16, tag="m16b", name="m16b")
            m16f = pp.tile([128, 16], F32, tag="m16f", name="m16f")
            mTf = pp.tile([16, 128], F32, tag="mTf", name="mTf")
            ones = pp.tile([128, 1], BF16, tag="ones", name="ones")
            i16p = pp.tile([128, 16], BF16, tag="i16p", name="i16p")
            nc.sync.dma_start(m16b[:], m16b_d[:, :])
            nc.sync.dma_start(m16f[:], m16f_d[:, :])
            nc.sync.dma_start(mTf[:], mTf_d[:, :])
            nc.sync.dma_start(ones[:], ones_d[:, :])
            nc.sync.dma_start(i16p[:], i16p_d[:, :])

            g2 = [pp.tile([128, FH], BF16, tag=f"g2_{m2}", name=f"g2_{m2}")
                  for m2 in range(2)]
            whs = [pp.tile([128, FH], BF16, tag=f"wh{k}", name=f"wh{k}")
                   for k in range(KH)]
            for k in range(KH):          # preload overlaps phase 1
                nc.sync.dma_start(whs[k][:], wh_d[128 * k : 128 * (k + 1), :])

            # ---------- state ---------------------------------------------
            hT2x = [st.tile([128, CH], BF16, tag=f"hT2x{i}", name=f"hT2x{i}")
                    for i in range(2)]
            nc.sync.dma_start(hT2x[0][:], h0t2_d[:, :])
            Dg = st.tile([128, CH], F32, tag="Dg", name="Dg")      # D = 2c
            nc.sync.dma_start(Dg[:], c0g_d[:, :])
            xwTs = [st.tile([128, FH], BF16, tag=f"xwT{i}", name=f"xwT{i}")
                    for i in range(2)]
            nc.vector.memset(xwTs[0][:], 0.0)
            nc.vector.memset(xwTs[1][:], 0.0)

            # ---------- phase 1: xwT = x @ Wx + b (bf16, t-major) ---------
            with tc.tile_pool(name="ph1c", bufs=1) as p1, \
                 tc.tile_pool(name="ph1w", bufs=2) as p1w:
                xTs = [p1.tile([128, NT], BF16, tag=f"xT{k}", name=f"xT{k}")
                       for k in range(D // 128)]
                for k in range(D // 128):
                    nc.sync.dma_start(xTs[k][:], xT_d[128 * k : 128 * (k + 1), :])
                bcols = p1.tile([128, FH], F32, tag="bcols", name="bcols")
                nc.sync.dma_start(bcols[:], bcols_d[:, :])

                for cc in range(FH // CH):       # 16 col chunks of 320
                    wxc = [p1w.tile([128, CH], BF16, tag=f"wx{k}", name=f"wx{k}")
                           for k in range(D // 128)]
                    for k in range(D // 128):
                        nc.sync.dma_start(
                            wxc[k][:],
                            wx_d[128 * k : 128 * (k + 1), CH * cc : CH * (cc + 1)],
                        )
                    for m in range(NT // 128):   # t-major row tiles
                        ps = psA.tile([128, 512], F32, tag=f"a{m % 4}",
                                      name=f"a{m % 4}")
                        for k in range(D // 128):
                            nc.tensor.matmul(
                                ps[:, 0:CH],
                                xTs[k][:, 128 * m : 128 * (m + 1)],
                                wxc[k][:],
                                start=(k == 0),
                                stop=(k == D // 128 - 1),
                            )
                        xwcol = p1w.tile([128, CH], BF16, tag="xwc", name="xwc")
                        nc.vector.tensor_add(
                            xwcol[:], ps[:, 0:CH],
                            bcols[:, CH * cc : CH * (cc + 1)],
                        )
                        # dst[t, n, CH*cc+u] for t = 8m + (p//16), n = p%16
                        dst = bass.AP(
                            xwT_d[:, :, :].tensor,
                            (8 * m) * NL * FH + CH * cc,
                            [[NL * FH, 8], [FH, 16], [1, CH]],
                        )
                        nc.sync.dma_start(dst, xwcol[:])

            # ---------- phase 2: G2 = afp.T @ Wattn (wat streamed) --------
            with tc.tile_pool(name="ph2w", bufs=2) as p2w:
                for jj in range(FH // 512):
                    watc = [p2w.tile([128, 512], BF16, tag=f"wat{k}",
                                     name=f"wat{k}") for k in range(KH)]
                    for k in range(KH):
                        nc.sync.dma_start(
                            watc[k][:],
                            wat_d[128 * k : 128 * (k + 1),
                                  512 * jj : 512 * (jj + 1)],
                        )
                    for m2 in range(2):
                        ps = psA.tile([128, 512], F32,
                                      tag=f"a{(2 * jj + m2) % 4}",
                                      name=f"a{(2 * jj + m2) % 4}")
                        for k in range(KH):
                            nc.tensor.matmul(
                                ps[:],
                                afp[:, 256 * k + 128 * m2 : 256 * k + 128 * (m2 + 1)],
                                watc[k][:],
                                start=(k == 0),
                                stop=(k == KH - 1),
                            )
                        if m2 == 0:
                            nc.vector.tensor_copy(
                                g2[m2][:, 512 * jj : 512 * (jj + 1)], ps[:])
                        else:
                            nc.scalar.activation(
                                g2[m2][:, 512 * jj : 512 * (jj + 1)], ps[:],
                                act.Copy)

            # ---------- phase 3: recurrence -------------------------------
            wk_cm = tc.tile_pool(name="wk", bufs=2)
            wk = wk_cm.__enter__()
            pkp_cm = tc.tile_pool(name="pkp", bufs=2)
            pkp = pkp_cm.__enter__()

            psa = [psA.tile([128, 512], F32, tag=f"a{r}", name=f"a{r}")
                   for r in range(4)]
            psm0 = psS.tile([128, 8], F32, tag="psm0", name="psm0")
            psm1 = psS.tile([128, 8], F32, tag="psm1", name="psm1")
            psjk = psS.tile([128, 512], F32, tag="psjk", name="psjk")

            nc.sync.dma_start(xwTs[0][0:16, :], xwT_d[0, :, :])

            def xw_add(r, xwT):
                for q in range(4):
                    cc = 4 * r + q
                    nc.tensor.matmul(
                        psa[r][32 * q : 32 * q + 16, 0:CH],
                        i16p[:, 0:16],
                        xwT[:, CH * cc : CH * (cc + 1)],
                        start=True, stop=False,
                        tile_position=(0, 32 * q),
                        skip_group_check=True,
                    )

            def hwh(r, hT, ks, stop):
                for k in ks:
                    for q in range(4):
                        cc = 4 * r + q
                        nc.tensor.matmul(
                            psa[r][32 * q : 32 * q + 16, 0:CH],
                            hT[:, 32 * k : 32 * k + 16],
                            whs[k][:, CH * cc : CH * (cc + 1)],
                            start=False,
                            stop=(stop and k == KH - 1),
                            tile_position=(0, 32 * q),
                            skip_group_check=True,
                        )

            def wg2(r, wsparse, stop):
                for m2 in range(2):
                    for q in range(4):
                        cc = 4 * r + q
                        nc.tensor.matmul(
                            psa[r][32 * q : 32 * q + 16, 0:CH],
                            wsparse[m2][:],
                            g2[m2][:, CH * cc : CH * (cc + 1)],
                            start=False,
                            stop=(stop and m2 == 1),
                            tile_position=(0, 32 * q),
                            skip_group_check=True,
                        )

            for t in range(t_steps):
                hT = hT2x[t % 2]
                hTn = hT2x[(t + 1) % 2]
                xwT = xwTs[t % 2]

                # ---- DVE: p2 = afp * (2h) broadcast over l ----
                p2t = pkp.tile([128, KH * L * NL], BF16, tag="pk", name="pk")
                pa = p2t[:]
                aa = afp[:]
                ha = hT[:]
                nc.vector.tensor_mul(
                    bass.AP(pa.tensor, pa.offset,
                            [pa.ap[0], [256, KH], [16, 16], [1, 16]]),
                    bass.AP(aa.tensor, aa.offset,
                            [aa.ap[0], [256, KH], [16, 16], [1, 16]]),
                    bass.AP(ha.tensor, ha.offset,
                            [ha.ap[0], [32, KH], [0, 16], [1, 16]]),
                )

                # ---- PE: f gate xw + h@Wh ----
                xw_add(GF, xwT)
                hwh(GF, hT, range(KH), stop=False)

                # ---- PE: attention scores (partition reduce via ones) ----
                ps_s = [psm0[:, 0:1], psm1[:, 0:1]]
                for k in range(KH):
                    for m2 in range(2):
                        nc.tensor.matmul(
                            ps_s[m2],
                            p2t[:, 256 * k + 128 * m2 : 256 * k + 128 * (m2 + 1)],
                            ones[:],
                            start=(k == 0),
                            stop=(k == KH - 1),
                            skip_group_check=True,
                        )
                expv = []
                for m2 in range(2):
                    e = wk.tile([128, 1], F32, tag=f"exp{m2}", name=f"exp{m2}")
                    nc.scalar.activation(e[:], ps_s[m2], act.Exp, scale=is2)
                    expv.append(e)

                # prefetch next step's xw
                if t + 1 < t_steps:
                    nc.sync.dma_start(
                        xwTs[(t + 1) % 2][0:16, :], xwT_d[t + 1, :, :])

                # ---- PE: i gate xw + h@Wh, den/bcast matmuls inserted ----
                xw_add(GI, xwT)
                hwh(GI, hT, range(0, 6), stop=False)
                ps_d = psm0[0:16, 4:5]
                for m2 in range(2):
                    nc.tensor.matmul(
                        ps_d, m16f[:], expv[m2][:],
                        start=(m2 == 0), stop=(m2 == 1),
                        skip_group_check=True,
                    )
                rden = wk.tile([16, 1], F32, tag="rden", name="rden")
                nc.vector.reciprocal(rden[:], ps_d)
                hwh(GI, hT, range(6, 9), stop=False)
                ps_r = psm0[:, 6:7]
                nc.tensor.matmul(ps_r, mTf[:], rden[:], start=True, stop=True,
                                 skip_group_check=True)
                hwh(GI, hT, range(9, KH), stop=False)

                # ---- DVE: softmax weights (sparse (l,n) layout) ----
                wsparse = []
                for m2 in range(2):
                    v = wk.tile([128, 1], BF16, tag=f"v{m2}", name=f"v{m2}")
                    nc.vector.tensor_mul(v[:], expv[m2][:], ps_r)
                    w_sp = wk.tile([128, 16], BF16, tag=f"wsp{m2}",
                                   name=f"wsp{m2}")
                    vb = v[:]
                    nc.vector.tensor_mul(
                        w_sp[:], m16b[:],
                        bass.AP(vb.tensor, vb.offset, [vb.ap[0], [0, 16]]),
                    )
                    wsparse.append(w_sp)

                # ---- PE: finish f and i gates with attn term ----
                wg2(GF, wsparse, stop=True)
                tf = wk.tile([128, CH], F32, tag="tf", name="tf")
                nc.scalar.activation(tf[:], psa[GF][:, 0:CH], act.Tanh,
                                     scale=0.5)
                wg2(GI, wsparse, stop=True)
                ti = wk.tile([128, CH], F32, tag="ti", name="ti")
                nc.scalar.activation(ti[:], psa[GI][:, 0:CH], act.Tanh,
                                     scale=0.5)

                # ---- PE: g gate ----
                xw_add(GG, xwT)
                wg2(GG, wsparse, stop=False)
                hwh(GG, hT, range(KH), stop=True)
                tg = wk.tile([128, CH], F32, tag="tg", name="tg")
                nc.scalar.activation(tg[:], psa[GG][:, 0:CH], act.Tanh)

                # ---- PE: o gate ----
                xw_add(GO, xwT)
                wg2(GO, wsparse, stop=False)
                hwh(GO, hT, range(KH), stop=True)

                # ---- c update: D = 2c;  D' = 0.5*(tf+1)*D + (ti+1)*tg ----
                t1 = wk.tile([128, CH], F32, tag="t1", name="t1")
                nc.vector.scalar_tensor_tensor(
                    t1[:], tf[:], 1.0, Dg[:], alu.add, alu.mult)
                t2 = wk.tile([128, CH], F32, tag="t2", name="t2")
                nc.vector.scalar_tensor_tensor(
                    t2[:], ti[:], 1.0, tg[:], alu.add, alu.mult)
                nc.vector.scalar_tensor_tensor(
                    Dg[:], t1[:], 0.5, t2[:], alu.mult, alu.add)
                tc_ = wk.tile([128, CH], F32, tag="tc", name="tc")
                nc.scalar.activation(tc_[:], Dg[:], act.Tanh, scale=0.5)

                # ---- tail: h = sig(o)*tanh(c); hb2 = 2h ----
                t_o = wk.tile([128, CH], F32, tag="t_o", name="t_o")
                nc.scalar.activation(t_o[:], psa[GO][:, 0:CH], act.Tanh,
                                     scale=0.5)
                hb2 = wk.tile([128, CH], BF16, tag="hb2", name="hb2")
                nc.vector.scalar_tensor_tensor(
                    hb2[:], t_o[:], 1.0, tc_[:], alu.add, alu.mult)
                nc.sync.dma_start(y_d[t, :, :], hb2[:])
                if t + 1 < t_steps:
                    nc.vector.transpose(hTn[:], hb2[:])

                # ---- PE keep-warm through the vector tail ----
                for _ in range(NJUNK):
                    nc.tensor.matmul(
                        psjk[0:16, 0:512], m16b[:, 0:16], whs[0][:, 0:512],
                        start=True, stop=True, skip_group_check=True,
                    )

            jout = wk.tile([1, 64], F32, tag="jout", name="jout")
            nc.vector.tensor_copy(jout[:], psjk[0:1, 0:64])
            nc.sync.dma_start(junk_d[:, :], jout[:])
            pkp_cm.__exit__(None, None, None)
            wk_cm.__exit__(None, None, None)

    nc.compile()
    return nc


_NC_CACHE = {}


def _get_nc(t_steps=T):
    if t_steps not in _NC_CACHE:
        _NC_CACHE[t_steps] = build_nc(t_steps)
    return _NC_CACHE[t_steps]


def _perm_cols(W):
    """Interleave gate columns: out[:, 320*(4g+q)+32b+c] = W[:, 1280g+128b+32q+c]."""
    Din = W.shape[0]
    return np.ascontiguousarray(
        W.reshape(Din, 4, 10, 4, 32).transpose(0, 1, 3, 2, 4).reshape(Din, FH)
    )


def _perm_vec(b):
    return np.ascontiguousarray(
        b.reshape(4, 10, 4, 32).transpose(0, 2, 1, 3).reshape(FH)
    )


def _prep_core_inputs(x, A, Wx, Wh, Wattn, b, c, t_steps=T):
    n0, n1 = NL * c, NL * (c + 1)
    NT = NL * t_steps
    xl = x[n0:n1]                                # (16, T, D)
    Afl = A[n0:n1].reshape(NL, H, L)             # (16, H, 16)
    h0 = Afl.mean(axis=-1).astype(np.float32)    # (16, H)

    # t-major: col = 16*t + n
    xT = np.ascontiguousarray(
        xl[:, :t_steps].transpose(2, 1, 0).reshape(D, NT))
    afp = np.ascontiguousarray(Afl.transpose(1, 2, 0).reshape(H, L * NL))
    afp = np.ascontiguousarray(
        afp.reshape(KH, 128, L * NL).transpose(1, 0, 2).reshape(128, KH * L * NL)
    )
    # hT carries 2h:  h0t2[p, 32k+n] = 2*h0[n, 128k+p]
    h0t2 = np.zeros((128, CH), np.float32)
    ht = 2.0 * h0.T  # (H, 16)
    for k in range(KH):
        h0t2[:, 32 * k : 32 * k + 16] = ht[128 * k : 128 * (k + 1), :]
    # D = 2c in interleaved gate layout: c0g[32q+n, 32b+c] = 2*h0[n, 128b+32q+c]
    c0g = np.zeros((128, CH), np.float32)
    t4 = (2.0 * h0).reshape(NL, 10, 4, 32).transpose(2, 0, 1, 3)
    c0g.reshape(4, 32, CH)[:, :16, :] = t4.reshape(4, NL, CH)

    p = np.arange(128)
    m16 = (p[:, None] % 16 == np.arange(16)[None, :]).astype(np.float32)
    i16 = (p[:, None] == np.arange(16)[None, :]).astype(np.float32)
    bcols = np.broadcast_to(
        _perm_vec(np.asarray(b, np.float32)), (128, FH)).copy()

    bf = _BF16_NP
    return {
        "xT": xT.astype(bf),
        "afp": afp.astype(bf),
        "wx": _perm_cols(np.asarray(Wx, np.float32)).astype(bf),
        "wh": (0.5 * _perm_cols(np.asarray(Wh, np.float32))).astype(bf),
        "wat": _perm_cols(np.asarray(Wattn, np.float32)).astype(bf),
        "bcols": bcols,
        "h0t2": h0t2.astype(bf),
        "c0g": c0g,
        "m16b": m16.astype(bf),
        "m16f": m16,
        "mTf": np.ascontiguousarray(m16.T),
        "ones": np.ones((128, 1), bf),
        "i16p": i16.astype(bf),
    }


def _run(x, A, Wx, Wh, Wattn, b, t_steps=T, trace=False):
    nc = _get_nc(t_steps)
    x = np.asarray(x, np.float32)
    A = np.asarray(A, np.float32)
    in_maps = [
        _prep_core_inputs(x, A, Wx, Wh, Wattn, b, c, t_steps)
        for c in range(N_CORES)
    ]
    kw = {}
    if trace:
        import types
        try:
            import antenv.axon_hooks  # noqa: F401
        except ImportError:
            from trn_agent_boot.trn_boot import _ntff_profile_via_ctypes
            hook = _ntff_profile_via_ctypes("/opt/axon/libaxon_pjrt.so")
            mod = types.ModuleType("antenv.axon_hooks")
            mod.get_axon_ntff_profile_hook = lambda: hook
            sys.modules["antenv.axon_hooks"] = mod
        kw["trace"] = True
    res = run_bass_kernel_spmd(nc, in_maps, core_ids=list(range(N_CORES)), **kw)
    outs = []
    for r in res.results:
        # y[t, 32q+n, 32b+c] = 2*h[n, t, 128b+32q+c]
        y = 0.5 * np.asarray(r["y"], np.float32)
        y5 = y.reshape(t_steps, 4, 32, KH, 32)[:, :, :NL, :, :]
        outs.append(np.ascontiguousarray(
            y5.transpose(2, 0, 3, 1, 4).reshape(NL, t_steps, H)))
    return np.concatenate(outs, axis=0), res.exec_time_ns


def kernel(x, A, Wx, Wh, Wattn, b):
    out, _ = _run(x, A, Wx, Wh, Wattn, b)
    return out
